# revision 30
# baseline (speedup 1.0000x reference)
"""DiT graph-attention block on 8 trn2 NeuronCores.

Sharding: core ci owns nodes [ci*5120, (ci+1)*5120). Phase A computes
q/k/v/u + adaLN for LOCAL nodes only, then an HBM AllGather replicates the
packed k|v|u table to every core. Edges are partitioned by dst owner,
sorted by dst, chunked into 128-dst windows; segment softmax and
scatter-add are device-local via indicator matmuls; src-side k/v/u rows
are fetched with dma_gather from the gathered table (int16 indices, so
the table is split at row 32768). All scalar-engine activations use only
{Exp, Ln, Abs, Copy, Identity, Square} (one act-table set, no reloads);
silu/sigmoid/tanh-gate/gelu are computed via exp + vector reciprocal, and
rsqrt via exp(-0.5*ln(var+eps)).
"""
import numpy as np

N, E, D, HEADS, HD, REL, ED, MLPH = 40000, 480000, 128, 8, 16, 64, 32, 512
NC_ = 8
NPAD = 40960          # padded node count (8 * 5120)
NLOC = NPAD // NC_    # 5120 local nodes per core
NCHUNK = NLOC // 128  # 40 chunks of 128 local nodes
LOCFM = NLOC // 512   # 10 feature-major groups of 512 nodes
HALF = 32768          # int16 index limit for dma_gather
KW = 384              # kvu row: k|v|u (256B-multiple rows for dma_gather)
QW = 256              # qu row:  q|u|pad (256B-multiple rows)


def _pack_idx16(idx_flat):
    """dma_gather int16 index layout: i -> [i%16, i//16], replicated x8."""
    n = len(idx_flat)
    a = np.zeros((16, n // 16), np.int16)
    a[np.arange(n) % 16, np.arange(n) // 16] = idx_flat
    return np.tile(a, (8, 1))


def _host_pack(edge_index):
    """Per-core edge packing (global node ids). Returns tile counts + aux."""
    src_g, dst_g = edge_index[0].astype(np.int64), edge_index[1].astype(np.int64)
    per_core = []
    for ci in range(NC_):
        base = ci * NLOC
        m = (dst_g >= base) & (dst_g < base + NLOC)
        s = src_g[m]
        d = dst_g[m] - base
        order = np.argsort(d, kind="stable")
        s, d = s[order], d[order]
        bounds = np.searchsorted(d, np.arange(0, NLOC + 1, 128))
        chunks = []
        for ch in range(NCHUNK):
            a, b = bounds[ch], bounds[ch + 1]
            sl, dl = s[a:b], d[a:b]
            lo = sl < HALF
            chunks.append(((sl[lo], dl[lo]), (sl[~lo], dl[~lo])))
        per_core.append(chunks)
    tlo = max(max(max((len(c[0][0]) + 127) // 128, 1) for c in chunks)
              for chunks in per_core)
    thi = max(max(max((len(c[1][0]) + 127) // 128, 1) for c in chunks)
              for chunks in per_core)
    aux = []
    TT = tlo + thi
    for ci in range(NC_):
        slo = np.zeros((NCHUNK, tlo * 128), np.int64)
        shi = np.zeros((NCHUNK, thi * 128), np.int64)
        sd = np.zeros((NCHUNK, TT * 128), np.int64)
        dw = np.full((NCHUNK, TT * 128), -1.0, np.float32)
        for ch in range(NCHUNK):
            (sl, dl), (sh, dh) = per_core[ci][ch]
            slo[ch, :len(sl)] = sl
            shi[ch, :len(sh)] = sh - HALF
            sd[ch, :len(sl)] = dl
            sd[ch, tlo * 128:tlo * 128 + len(sh)] = dh
            dw[ch, :len(sl)] = dl - ch * 128
            dw[ch, tlo * 128:tlo * 128 + len(sh)] = dh - ch * 128
        slo16 = np.concatenate([_pack_idx16(slo[ch].astype(np.int16))
                                for ch in range(NCHUNK)], axis=1)
        shi16 = np.concatenate([_pack_idx16(shi[ch].astype(np.int16))
                                for ch in range(NCHUNK)], axis=1)
        sd16 = np.concatenate([_pack_idx16(sd[ch].astype(np.int16))
                               for ch in range(NCHUNK)], axis=1)
        dwin = dw.reshape(NCHUNK * TT, 128).T.copy()  # [128, NCHUNK*TT]
        aux.append(dict(slo16=slo16, shi16=shi16, sd16=sd16, dwin=dwin))
    return tlo, thi, aux


def _build(TLO, THI, no_cc=False):
    import concourse.bass as bass
    import concourse.bacc as bacc
    import concourse.mybir as mybir
    from concourse.tile import TileContext
    _f32, _bf16 = mybir.dt.float32, mybir.dt.bfloat16
    AF = mybir.ActivationFunctionType
    OP = mybir.AluOpType
    TT = TLO + THI

    nc = bacc.Bacc("TRN2", target_bir_lowering=False, debug=False,
                   num_devices=NC_)
    din = {}

    def I(name, shape, dt=None):
        din[name] = nc.dram_tensor(name, shape, dt or _f32,
                                   kind="ExternalInput")
        return din[name]

    x_in = I("x", [NLOC, D])
    c_in = I("c", [NLOC, D])
    for nm, sh in [("wq", [D, D]), ("wk", [D, D]), ("wv", [D, D]),
                   ("wp", [D, D]), ("wrel", [D, REL]), ("wada", [D, 6 * D]),
                   ("w1e", [2 * ED, 3 * 2 * ED]), ("w2e", [2 * ED, ED]),
                   ("wbg", [ED, 2 * HEADS]), ("wf1", [D, MLPH]),
                   ("wf2", [D, MLPH]), ("ones", [128, 128]),
                   ("identb", [128, 128])]:
        I(nm, sh, _bf16)
    I("identf", [128, 128], _f32)
    I("iota", [128, 128], _f32)
    I("slo16", [128, NCHUNK * TLO * 8], mybir.dt.int16)
    I("shi16", [128, NCHUNK * THI * 8], mybir.dt.int16)
    I("sd16", [128, NCHUNK * TT * 8], mybir.dt.int16)
    I("dwin", [128, NCHUNK * TT], _f32)
    y_out = nc.dram_tensor("y", [NLOC, D], _f32, kind="ExternalOutput")

    scale = float(HD) ** -0.5

    with TileContext(nc) as tc:
        with (tc.tile_pool(name="const", bufs=1) as cp,
              tc.tile_pool(name="pers", bufs=1) as pp,
              tc.tile_pool(name="dram", bufs=1, space="DRAM") as dp,
              tc.tile_pool(name="work", bufs=2) as wp_,
              tc.tile_pool(name="work2", bufs=2) as wp2,
              tc.tile_pool(name="ps", bufs=3, space="PSUM") as ps,
              tc.tile_pool(name="ps2", bufs=2, space="PSUM") as ps2):

            # ---- constants / weights into SBUF
            W = {}
            for nm in ["wq", "wk", "wv", "wp", "wrel", "wada", "w1e", "w2e",
                       "wbg", "wf1", "wf2", "ones", "identb", "identf",
                       "iota"]:
                t = cp.tile(list(din[nm].shape),
                            _f32 if nm in ("identf", "iota") else _bf16,
                            tag=nm)
                nc.sync.dma_start(out=t[:], in_=din[nm][:, :])
                W[nm] = t
            aux = {}
            for nm in ["slo16", "shi16", "sd16"]:
                t = cp.tile(list(din[nm].shape), mybir.dt.int16, tag=nm)
                nc.sync.dma_start(out=t[:], in_=din[nm][:, :])
                aux[nm] = t
            dwin_sb = cp.tile([128, NCHUNK * TT], _f32)
            nc.sync.dma_start(out=dwin_sb[:], in_=din["dwin"][:, :])
            CONSTS = {"eps": 1e-6, "iD": 1.0 / D, "iR": 1.0 / REL,
                      "nh": -0.5, "n1": -1.0, "n2": -2.0, "ng": -1.702}
            C = {}
            for nm, v in CONSTS.items():
                t = cp.tile([128, 1], _f32, tag="c_" + nm)
                nc.gpsimd.memset(t[:], v)
                C[nm] = t

            kvu_loc = dp.tile([NLOC, KW], _bf16)
            kvu_t = dp.tile([NPAD, KW], _bf16, addr_space="Shared")
            qu_t = dp.tile([NLOC, QW], _bf16)

            # persistent local fm tables
            gm_t = pp.tile([128, NLOC], _bf16)
            scm_t = pp.tile([128, NLOC], _bf16)
            shm_t = pp.tile([128, NLOC], _bf16)
            gml_t = pp.tile([128, NLOC], _bf16)

            # ======== PHASE A: local node phase ========
            for g in range(LOCFM):
                r0 = g * 512
                ln_fm = wp_.tile([128, 512], _bf16, tag="lnfm")
                scfm = wp_.tile([128, 512], _bf16, tag="scfm")
                for j in range(4):
                    rr = r0 + j * 128
                    xe = wp_.tile([128, 128], _f32, tag="xe")
                    nc.sync.dma_start(out=xe[:], in_=x_in[rr:rr + 128, :])
                    ce = wp_.tile([128, 128], _f32, tag="ce")
                    nc.sync.dma_start(out=ce[:], in_=c_in[rr:rr + 128, :])
                    # LN stats per node (free-dim)
                    s1 = wp_.tile([128, 1], _f32, tag="s1")
                    xb = wp_.tile([128, 128], _bf16, tag="xb")
                    nc.scalar.activation(xb[:], xe[:], AF.Copy, accum_out=s1[:])
                    sq = wp_.tile([128, 128], _bf16, tag="sq")
                    s2 = wp_.tile([128, 1], _f32, tag="s2")
                    nc.vector.scalar_tensor_tensor(
                        out=sq[:], in0=xe[:], scalar=1.0, in1=xe[:],
                        op0=OP.mult, op1=OP.mult, accum_out=s2[:])
                    mean = wp_.tile([128, 1], _f32, tag="mean")
                    nc.scalar.activation(mean[:], s1[:], AF.Copy, scale=C["iD"][:])
                    msq = wp_.tile([128, 1], _f32, tag="msq")
                    nc.vector.tensor_mul(out=msq[:], in0=mean[:], in1=mean[:])
                    var = wp_.tile([128, 1], _f32, tag="var")
                    nc.vector.scalar_tensor_tensor(
                        out=var[:], in0=s2[:], scalar=1. / D, in1=msq[:],
                        op0=OP.mult, op1=OP.subtract)
                    lnv = wp_.tile([128, 1], _f32, tag="lnv")
                    nc.scalar.activation(lnv[:], var[:], AF.Ln, bias=C["eps"][:])
                    rstd = wp_.tile([128, 1], _f32, tag="rstd")
                    nc.scalar.activation(rstd[:], lnv[:], AF.Exp, scale=C["nh"][:])
                    nmr = wp_.tile([128, 1], _f32, tag="nmr")
                    nc.vector.scalar_tensor_tensor(
                        out=nmr[:], in0=mean[:], scalar=-1.0, in1=rstd[:],
                        op0=OP.mult, op1=OP.mult)
                    lnem = wp_.tile([128, 128], _bf16, tag="lnem")
                    nc.scalar.activation(lnem[:], xe[:], AF.Identity,
                                         scale=rstd[:], bias=nmr[:])
                    pt = ps.tile([128, 128], _bf16, tag="sm")
                    nc.tensor.transpose(pt[:], lnem[:], W["identb"][:])
                    nc.vector.tensor_copy(out=ln_fm[:, j * 128:(j + 1) * 128],
                                          in_=pt[:])
                    # silu(c) = c * sigmoid(c), sigmoid via exp + reciprocal
                    ces = wp_.tile([128, 128], _bf16, tag="ces")
                    nc.scalar.activation(ces[:], ce[:], AF.Exp, scale=C["n1"][:])
                    cden = wp_.tile([128, 128], _bf16, tag="cden")
                    nc.vector.tensor_scalar_add(out=cden[:], in0=ces[:],
                                                scalar1=1.0)
                    crec = wp_.tile([128, 128], _bf16, tag="crec")
                    with nc.allow_low_precision(reason="sigmoid recip"):
                        nc.vector.reciprocal(out=crec[:], in_=cden[:])
                    sce = wp_.tile([128, 128], _bf16, tag="sce")
                    nc.vector.tensor_mul(out=sce[:], in0=ce[:], in1=crec[:])
                    pt2 = ps.tile([128, 128], _bf16, tag="sm")
                    nc.tensor.transpose(pt2[:], sce[:], W["identb"][:])
                    nc.vector.tensor_copy(out=scfm[:, j * 128:(j + 1) * 128],
                                          in_=pt2[:])
                # modulate: h = ln * (1 + sc_msa) + sh_msa
                pa_sc = ps.tile([128, 512], _f32, tag="big")
                nc.tensor.matmul(pa_sc[:], W["wada"][:, 128:256], scfm[:],
                                 start=True, stop=True)
                pa_sh = ps.tile([128, 512], _f32, tag="big")
                nc.tensor.matmul(pa_sh[:], W["wada"][:, 0:128], scfm[:],
                                 start=True, stop=True)
                t3 = wp_.tile([128, 512], _bf16, tag="t3")
                nc.vector.scalar_tensor_tensor(
                    out=t3[:], in0=pa_sc[:], scalar=1.0, in1=ln_fm[:],
                    op0=OP.add, op1=OP.mult)
                h_bf = wp_.tile([128, 512], _bf16, tag="hbf")
                nc.vector.tensor_add(out=h_bf[:], in0=t3[:], in1=pa_sh[:])
                # k, v
                stage = wp2.tile([128, 4, KW], _bf16, tag="stage",
                                 bufs=1)
                for nm, off in [("wk", 0), ("wv", 128)]:
                    pk = ps.tile([128, 512], _f32, tag="big")
                    nc.tensor.matmul(pk[:], W[nm][:], h_bf[:], start=True,
                                     stop=True)
                    ksb = wp_.tile([128, 512], _bf16, tag="ksb")
                    nc.scalar.activation(ksb[:], pk[:], AF.Copy)
                    for j in range(4):
                        ptk = ps.tile([128, 128], _bf16, tag="sm")
                        nc.tensor.transpose(
                            ptk[:], ksb[:, j * 128:(j + 1) * 128],
                            W["identb"][:])
                        nc.vector.tensor_copy(
                            out=stage[:, j, off:off + 128], in_=ptk[:])
                # u: rel proj + LN + store
                pu = ps.tile([64, 512], _f32, tag="big")
                nc.tensor.matmul(pu[:], W["wrel"][:], h_bf[:], start=True,
                                 stop=True)
                usb = wp_.tile([64, 512], _bf16, tag="usb")
                nc.scalar.activation(usb[:], pu[:], AF.Copy)
                for j in range(4):
                    put = ps.tile([128, 64], _bf16, tag="sm")
                    nc.tensor.transpose(put[:], usb[:, j * 128:(j + 1) * 128],
                                        W["identb"][:64, :64])
                    us1 = wp_.tile([128, 1], _f32, tag="us1")
                    ue = wp_.tile([128, 64], _f32, tag="ue")
                    nc.scalar.activation(ue[:], put[:], AF.Copy,
                                         accum_out=us1[:])
                    usq = wp_.tile([128, 64], _bf16, tag="usq")
                    us2 = wp_.tile([128, 1], _f32, tag="us2")
                    nc.vector.scalar_tensor_tensor(
                        out=usq[:], in0=ue[:], scalar=1.0, in1=ue[:],
                        op0=OP.mult, op1=OP.mult, accum_out=us2[:])
                    um = wp_.tile([128, 1], _f32, tag="um")
                    nc.scalar.activation(um[:], us1[:], AF.Copy,
                                         scale=C["iR"][:])
                    umq = wp_.tile([128, 1], _f32, tag="umq")
                    nc.vector.tensor_mul(out=umq[:], in0=um[:], in1=um[:])
                    uva = wp_.tile([128, 1], _f32, tag="uva")
                    nc.vector.scalar_tensor_tensor(
                        out=uva[:], in0=us2[:], scalar=1. / REL, in1=umq[:],
                        op0=OP.mult, op1=OP.subtract)
                    ulnv = wp_.tile([128, 1], _f32, tag="ulnv")
                    nc.scalar.activation(ulnv[:], uva[:], AF.Ln, bias=C["eps"][:])
                    urs = wp_.tile([128, 1], _f32, tag="urs")
                    nc.scalar.activation(urs[:], ulnv[:], AF.Exp, scale=C["nh"][:])
                    unm = wp_.tile([128, 1], _f32, tag="unm")
                    nc.vector.scalar_tensor_tensor(
                        out=unm[:], in0=um[:], scalar=-1.0, in1=urs[:],
                        op0=OP.mult, op1=OP.mult)
                    nc.scalar.activation(stage[:, j, 256:320], put[:],
                                         AF.Identity, scale=urs[:], bias=unm[:])
                # q + local qu table
                qstage = wp2.tile([128, 4, QW], _bf16, tag="qstage",
                                  bufs=1)
                pq = ps.tile([128, 512], _f32, tag="big")
                nc.tensor.matmul(pq[:], W["wq"][:], h_bf[:], start=True,
                                 stop=True)
                qsb = wp_.tile([128, 512], _bf16, tag="qsb")
                nc.scalar.activation(qsb[:], pq[:], AF.Copy)
                for j in range(4):
                    ptq = ps.tile([128, 128], _bf16, tag="sm")
                    nc.tensor.transpose(
                        ptq[:], qsb[:, j * 128:(j + 1) * 128], W["identb"][:])
                    nc.vector.tensor_copy(out=qstage[:, j, 0:128], in_=ptq[:])
                    nc.vector.tensor_copy(out=qstage[:, j, 128:192],
                                          in_=stage[:, j, 256:320])
                nc.gpsimd.dma_start(
                    out=qu_t[r0:r0 + 512, :].rearrange(
                        "(j p) f -> p j f", p=128),
                    in_=qstage[:])
                # ada: g_msa(2), sh_mlp(3), sc_mlp(4), g_mlp(5)
                for wsl, dst_t in [(2, gm_t), (4, scm_t), (3, shm_t),
                                   (5, gml_t)]:
                    pad = ps.tile([128, 512], _f32, tag="big")
                    nc.tensor.matmul(
                        pad[:], W["wada"][:, wsl * 128:(wsl + 1) * 128],
                        scfm[:], start=True, stop=True)
                    nc.scalar.activation(dst_t[:, r0:r0 + 512], pad[:],
                                         AF.Copy)
                nc.gpsimd.dma_start(
                    out=kvu_loc[r0:r0 + 512, :].rearrange(
                        "(j p) f -> p j f", p=128),
                    in_=stage[:])

            # ======== AllGather the kvu table ========
            if no_cc:
                # TimelineSim can't model collectives; stand in a same-size
                # local copy so the rest of the schedule is representative.
                nc.gpsimd.dma_start(out=kvu_t[0:NLOC, :], in_=kvu_loc[:])
            else:
                nc.gpsimd.collective_compute(
                    "AllGather", mybir.AluOpType.bypass,
                    replica_groups=[list(range(NC_))],
                    ins=[kvu_loc.opt()], outs=[kvu_t.opt()])

            # ======== PHASE B: edge phase ========
            for ch in range(NCHUNK):
                acc = ps2.tile([128, 136], _f32, tag="acc")
                kvg = wp2.tile([128, TT, KW], _bf16, tag="kvg")
                nc.gpsimd.dma_gather(
                    out_ap=kvg[:, 0:TLO, :], in_ap=kvu_t[0:HALF, :],
                    idxs_ap=aux["slo16"][:, ch * TLO * 8:(ch + 1) * TLO * 8],
                    num_idxs=TLO * 128, num_idxs_reg=TLO * 128,
                    elem_size=KW, single_packet=False)
                nc.gpsimd.dma_gather(
                    out_ap=kvg[:, TLO:TT, :], in_ap=kvu_t[HALF:NPAD, :],
                    idxs_ap=aux["shi16"][:, ch * THI * 8:(ch + 1) * THI * 8],
                    num_idxs=THI * 128, num_idxs_reg=THI * 128,
                    elem_size=KW, single_packet=False)
                qug = wp2.tile([128, TT, QW], _bf16, tag="qug")
                nc.gpsimd.dma_gather(
                    out_ap=qug[:], in_ap=qu_t[:, :],
                    idxs_ap=aux["sd16"][:, ch * TT * 8:(ch + 1) * TT * 8],
                    num_idxs=TT * 128, num_idxs_reg=TT * 128, elem_size=QW,
                    single_packet=False)
                # batched em ops over all TT tiles
                tqk = wp2.tile([128, TT, 128], _bf16, tag="tqk",
                               bufs=2)
                nc.vector.tensor_mul(out=tqk[:], in0=kvg[:, :, 0:128],
                                     in1=qug[:, :, 0:128])
                sim = wp2.tile([128, TT, 8], _f32, tag="sim", bufs=2)
                nc.vector.tensor_reduce(
                    out=sim[:], in_=tqk[:].rearrange("p t (h d) -> p t h d",
                                                     h=8),
                    axis=mybir.AxisListType.X, op=OP.add)
                # u_i|u_j side-by-side so one 128-wide transpose covers both
                uu_em = wp2.tile([128, TT, 128], _bf16, tag="uuem", bufs=2)
                nc.gpsimd.tensor_copy(out=uu_em[:, :, 0:64],
                                      in_=qug[:, :, 128:192])
                nc.gpsimd.tensor_copy(out=uu_em[:, :, 64:128],
                                      in_=kvg[:, :, 256:320])
                bg_em = wp2.tile([128, TT, 16], _bf16, tag="bgem",
                                 bufs=2)
                # edge MLP in sub-batches of 4 tiles (512 edges)
                for b0 in range(0, TT, 4):
                    bn = min(4, TT - b0)
                    wd = bn * 128
                    pT = ps.tile([64, 1024], _bf16, tag="big")
                    for i in range(bn):
                        nc.tensor.transpose(
                            pT[:, i * 128:(i + 1) * 128],
                            uu_em[:, b0 + i, 0:64], W["identb"][:])
                        nc.tensor.transpose(
                            pT[:, 512 + i * 128:512 + (i + 1) * 128],
                            uu_em[:, b0 + i, 64:128], W["identb"][:])
                    fmuu = wp_.tile([64, 1024], _bf16, tag="fmuu")
                    nc.scalar.activation(fmuu[:], pT[:], AF.Copy)
                    adf = wp_.tile([64, 512], _bf16, tag="adf")
                    nc.gpsimd.tensor_tensor(out=adf[:, :wd],
                                            in0=fmuu[:, :wd],
                                            in1=fmuu[:, 512:512 + wd],
                                            op=OP.subtract)
                    nc.scalar.activation(adf[:, :wd], adf[:, :wd], AF.Abs)
                    pe1 = ps.tile([64, 512], _f32, tag="big")
                    nc.tensor.matmul(pe1[:, :wd], W["w1e"][:, 0:64],
                                     fmuu[:, :wd], start=True, stop=False)
                    nc.tensor.matmul(pe1[:, :wd], W["w1e"][:, 64:128],
                                     fmuu[:, 512:512 + wd], start=False,
                                     stop=False)
                    nc.tensor.matmul(pe1[:, :wd], W["w1e"][:, 128:192],
                                     adf[:, :wd], start=False, stop=True)
                    # silu via exp + divide
                    es = wp_.tile([64, 512], _bf16, tag="es")
                    nc.scalar.activation(es[:, :wd], pe1[:, :wd], AF.Exp,
                                         scale=C["n1"][:64])
                    edn = wp_.tile([64, 512], _bf16, tag="edn")
                    nc.vector.tensor_scalar_add(out=edn[:, :wd],
                                                in0=es[:, :wd], scalar1=1.0)
                    erc = wp_.tile([64, 512], _bf16, tag="erc")
                    with nc.allow_low_precision(reason="sigmoid recip"):
                        nc.vector.reciprocal(out=erc[:, :wd],
                                             in_=edn[:, :wd])
                    ef1 = wp_.tile([64, 512], _bf16, tag="ef1")
                    nc.vector.tensor_mul(out=ef1[:, :wd], in0=pe1[:, :wd],
                                         in1=erc[:, :wd])
                    pe2 = ps.tile([32, 512], _f32, tag="big")
                    nc.tensor.matmul(pe2[:, :wd], W["w2e"][:], ef1[:, :wd],
                                     start=True, stop=True)
                    ef2 = wp_.tile([32, 512], _bf16, tag="ef2")
                    nc.scalar.activation(ef2[:, :wd], pe2[:, :wd], AF.Copy)
                    # bias/gate straight to em: ef2 tile-slice as stationary
                    pbe = ps.tile([128, 64], _f32, tag="sm")
                    for i in range(bn):
                        nc.tensor.matmul(
                            pbe[:, i * 16:(i + 1) * 16],
                            ef2[:, i * 128:(i + 1) * 128], W["wbg"][:],
                            start=True, stop=True)
                    nc.scalar.activation(
                        bg_em[:, b0:b0 + bn, :],
                        pbe[:, :bn * 16].rearrange("p (t f) -> p t f", f=16),
                        AF.Copy)
                # batched weights/gates over all TT tiles
                sb_ = wp_.tile([128, TT, 8], _f32, tag="sb_")
                nc.vector.scalar_tensor_tensor(
                    out=sb_[:], in0=sim[:], scalar=scale,
                    in1=bg_em[:, :, 0:8], op0=OP.mult, op1=OP.add)
                w_ = wp_.tile([128, TT, 8], _bf16, tag="w_")
                nc.scalar.activation(w_[:], sb_[:], AF.Exp)
                # gate: 1 + tanh(g) = 2*sigmoid(2g); the 2 is folded into Wp
                gs = wp_.tile([128, TT, 8], _bf16, tag="gs")
                nc.scalar.activation(gs[:], bg_em[:, :, 8:16], AF.Exp,
                                     scale=C["n2"][:])
                gdn = wp_.tile([128, TT, 8], _bf16, tag="gdn")
                nc.vector.tensor_scalar_add(out=gdn[:], in0=gs[:], scalar1=1.0)
                grc = wp_.tile([128, TT, 8], _bf16, tag="grc")
                with nc.allow_low_precision(reason="sigmoid recip"):
                    nc.vector.reciprocal(out=grc[:], in_=gdn[:])
                wsg = wp_.tile([128, TT, 8], _bf16, tag="wsg")
                nc.vector.tensor_mul(out=wsg[:], in0=w_[:], in1=grc[:])
                msgw = wp2.tile([128, TT, 136], _bf16, tag="msgw",
                                bufs=2)
                nc.vector.tensor_mul(
                    out=msgw[:, :, 0:128].rearrange("p t (h d) -> p t h d",
                                                    h=8),
                    in0=kvg[:, :, 128:256].rearrange("p t (h d) -> p t h d",
                                                     h=8),
                    in1=wsg[:, :, :, None].to_broadcast([128, TT, 8, 16]))
                nc.vector.tensor_copy(out=msgw[:, :, 128:136], in_=w_[:])
                for t in range(TT):
                    gt = ch * TT + t
                    ind = wp_.tile([128, 128], _bf16, tag="ind")
                    nc.gpsimd.tensor_scalar(
                        out=ind[:], in0=W["iota"][:],
                        scalar1=dwin_sb[:, gt:gt + 1], scalar2=None,
                        op0=OP.is_equal)
                    nc.tensor.matmul(acc[:], ind[:], msgw[:, t, :],
                                     start=(t == 0), stop=(t == TT - 1))
                # ---- fused close over chunk pairs (256 nodes)
                if ch % 2 == 0:
                    acc_prev = acc
                    continue
                accA, accB = acc_prev, acc
                co = (ch - 1) * 128
                agg = wp_.tile([128, 2, 8, 16], _bf16, tag="agg")
                for i, a_ in enumerate((accA, accB)):
                    de = wp_.tile([128, 8], _f32, tag="de")
                    nc.vector.tensor_scalar_add(out=de[:], in0=a_[:, 128:136],
                                                scalar1=1e-16)
                    r = wp_.tile([128, 8], _f32, tag="r")
                    nc.vector.reciprocal(out=r[:], in_=de[:])
                    nc.vector.tensor_mul(
                        out=agg[:, i],
                        in0=a_[:, 0:128].rearrange("p (h d) -> p h d", h=8),
                        in1=r[:, :, None].to_broadcast([128, 8, 16]))
                pag = ps.tile([128, 256], _bf16, tag="sm")
                for i in range(2):
                    nc.tensor.transpose(
                        pag[:, i * 128:(i + 1) * 128],
                        agg[:, i].rearrange("p h d -> p (h d)"),
                        W["identb"][:])
                agf = wp_.tile([128, 256], _bf16, tag="agf")
                nc.vector.tensor_copy(out=agf[:], in_=pag[:])
                pao = ps.tile([128, 256], _f32, tag="sm")
                nc.tensor.matmul(pao[:], W["wp"][:], agf[:], start=True,
                                 stop=True)
                t4 = wp_.tile([128, 256], _f32, tag="t4")
                nc.vector.tensor_mul(out=t4[:], in0=gm_t[:, co:co + 256],
                                     in1=pao[:])
                xe2 = wp_.tile([128, 2, 128], _f32, tag="xe2")
                nc.sync.dma_start(
                    out=xe2[:],
                    in_=x_in[co:co + 256, :].rearrange("(j p) d -> p j d",
                                                       p=128))
                pxf = ps.tile([128, 256], _f32, tag="sm")
                for i in range(2):
                    nc.tensor.transpose(pxf[:, i * 128:(i + 1) * 128],
                                        xe2[:, i, :], W["identf"][:])
                xu = wp_.tile([128, 256], _f32, tag="xu")
                nc.vector.tensor_add(out=xu[:], in0=pxf[:], in1=t4[:])
                # LN2 fm: one matmul computes sum(x) and sum(x^2)
                xusq = wp_.tile([128, 512], _bf16, tag="xusq")
                nc.vector.tensor_copy(out=xusq[:, 0:256], in_=xu[:])
                nc.scalar.activation(xusq[:, 256:512], xu[:], AF.Square)
                pst = ps.tile([1, 512], _f32, tag="sm")
                nc.tensor.matmul(pst[:], W["ones"][:, 0:1], xusq[:],
                                 start=True, stop=True)
                msum = wp_.tile([1, 512], _f32, tag="msum")
                nc.scalar.activation(msum[:], pst[:], AF.Copy,
                                     scale=C["iD"][:1])
                mq2 = wp_.tile([1, 256], _f32, tag="mq2")
                nc.vector.tensor_mul(out=mq2[:], in0=msum[:, 0:256],
                                     in1=msum[:, 0:256])
                vr2 = wp_.tile([1, 256], _f32, tag="vr2")
                nc.vector.tensor_sub(out=vr2[:], in0=msum[:, 256:512],
                                     in1=mq2[:])
                l2v = wp_.tile([1, 256], _f32, tag="l2v")
                nc.scalar.activation(l2v[:], vr2[:], AF.Ln, bias=C["eps"][:1])
                rs2 = wp_.tile([1, 256], _bf16, tag="rs2")
                nc.scalar.activation(rs2[:], l2v[:], AF.Exp, scale=C["nh"][:1])
                nm2 = wp_.tile([1, 256], _bf16, tag="nm2")
                nc.vector.scalar_tensor_tensor(
                    out=nm2[:], in0=msum[:, 0:256], scalar=-1.0, in1=rs2[:],
                    op0=OP.mult, op1=OP.mult)
                prb = ps.tile([128, 256], _f32, tag="sm")
                nc.tensor.matmul(prb[:], W["ones"][0:1, :], rs2[:],
                                 start=True, stop=True)
                pnb = ps.tile([128, 256], _f32, tag="sm")
                nc.tensor.matmul(pnb[:], W["ones"][0:1, :], nm2[:],
                                 start=True, stop=True)
                l1 = wp_.tile([128, 256], _bf16, tag="l1")
                nc.vector.tensor_mul(out=l1[:], in0=xusq[:, 0:256],
                                     in1=prb[:])
                l2 = wp_.tile([128, 256], _bf16, tag="l2")
                nc.vector.tensor_add(out=l2[:], in0=l1[:], in1=pnb[:])
                t5 = wp_.tile([128, 256], _bf16, tag="t5")
                nc.vector.scalar_tensor_tensor(
                    out=t5[:], in0=scm_t[:, co:co + 256], scalar=1.0,
                    in1=l2[:], op0=OP.add, op1=OP.mult)
                h2 = wp_.tile([128, 256], _bf16, tag="h2")
                nc.vector.tensor_add(out=h2[:], in0=t5[:],
                                     in1=shm_t[:, co:co + 256])
                pmo = ps.tile([128, 256], _f32, tag="sm")
                for jm in range(4):
                    pm1 = ps.tile([128, 256], _f32, tag="sm")
                    nc.tensor.matmul(pm1[:],
                                     W["wf1"][:, jm * 128:(jm + 1) * 128],
                                     h2[:], start=True, stop=True)
                    # gelu(x) ~= x*sigmoid(1.702x) via exp + divide
                    ms = wp_.tile([128, 256], _bf16, tag="ms")
                    nc.scalar.activation(ms[:], pm1[:], AF.Exp,
                                         scale=C["ng"][:])
                    mdn = wp_.tile([128, 256], _bf16, tag="mdn")
                    nc.vector.tensor_scalar_add(out=mdn[:], in0=ms[:],
                                                scalar1=1.0)
                    mrc = wp_.tile([128, 256], _bf16, tag="mrc")
                    with nc.allow_low_precision(reason="sigmoid recip"):
                        nc.vector.reciprocal(out=mrc[:], in_=mdn[:])
                    gl = wp_.tile([128, 256], _bf16, tag="gl")
                    nc.vector.tensor_mul(out=gl[:], in0=pm1[:], in1=mrc[:])
                    nc.tensor.matmul(pmo[:],
                                     W["wf2"][:, jm * 128:(jm + 1) * 128],
                                     gl[:], start=(jm == 0), stop=(jm == 3))
                t6 = wp_.tile([128, 256], _f32, tag="t6")
                nc.vector.tensor_mul(out=t6[:], in0=gml_t[:, co:co + 256],
                                     in1=pmo[:])
                yf = wp_.tile([128, 256], _f32, tag="yf")
                nc.vector.tensor_add(out=yf[:], in0=xu[:], in1=t6[:])
                pye = ps.tile([128, 256], _f32, tag="sm")
                for i in range(2):
                    nc.tensor.transpose(pye[:, i * 128:(i + 1) * 128],
                                        yf[:, i * 128:(i + 1) * 128],
                                        W["identf"][:])
                yem = wp_.tile([128, 2, 128], _f32, tag="yem")
                nc.vector.tensor_copy(out=yem[:], in_=pye[:].rearrange(
                    "p (j d) -> p j d", j=2))
                nc.scalar.dma_start(
                    out=y_out[co:co + 256, :].rearrange("(j p) d -> p j d",
                                                        p=128),
                    in_=yem[:])
    # Steer the act-table placement pass to the one set that holds every
    # function this kernel uses (exp, ln, abs, copy, identity, square):
    # hide exp/ln from the other sets during placement so it can't bounce
    # between exp-only and ln-only tables. Set ids stay positional, and the
    # chosen set really does contain exp+ln, so runtime tables are correct.
    import concourse.bacc as bacc_mod
    _orig_gat = bacc_mod.get_activation_tables

    def _gat_pinned(arch):
        tabs = _orig_gat(arch)
        drop = {mybir.ActivationFunctionType.Exp,
                mybir.ActivationFunctionType.Ln}
        return {name: (funcs if "natural_log_exp" in name
                       else funcs - drop)
                for name, funcs in tabs.items()}

    bacc_mod.get_activation_tables = _gat_pinned
    try:
        nc.compile()
    finally:
        bacc_mod.get_activation_tables = _orig_gat
    return nc


_CACHE = {}
LAST_RESULT = None


def kernel(**inputs):
    from concourse.bass_utils import run_bass_kernel_spmd

    x = np.asarray(inputs["x"], np.float32)
    c = np.asarray(inputs["c"], np.float32)
    ei = np.asarray(inputs["edge_index"])
    TLO, THI, aux = _host_pack(ei)

    import ml_dtypes

    def b16(a):
        return np.asarray(a, np.float32).astype(ml_dtypes.bfloat16)

    key = (TLO, THI)
    if key not in _CACHE:
        _CACHE[key] = _build(TLO, THI)
    nc = _CACHE[key]

    xp = np.zeros((NPAD, D), np.float32)
    xp[:N] = x
    cp_ = np.zeros((NPAD, D), np.float32)
    cp_[:N] = c
    ones = np.ones((128, 128), np.float32)
    ident = np.eye(128, dtype=np.float32)
    iota = np.tile(np.arange(128, dtype=np.float32), (128, 1))
    wbg = np.concatenate([inputs["Wbias"], inputs["Wgate"]], axis=1)

    common = dict(
        wq=b16(inputs["Wq"]), wk=b16(inputs["Wk"]), wv=b16(inputs["Wv"]),
        wp=b16(2.0 * np.asarray(inputs["Wp"], np.float32)),
        wrel=b16(inputs["Wrel"]),
        wada=b16(inputs["Wada"]),
        w1e=b16(np.concatenate([inputs["W1e"][0:64], inputs["W1e"][64:128],
                                inputs["W1e"][128:192]], axis=1)),
        w2e=b16(inputs["W2e"]), wbg=b16(wbg), wf1=b16(inputs["Wf1"]),
        wf2=b16(np.concatenate([inputs["Wf2"][i * 128:(i + 1) * 128]
                                for i in range(4)], axis=1)),
        ones=b16(ones), identb=b16(ident), identf=ident, iota=iota)

    in_maps = []
    for ci in range(NC_):
        lo = ci * NLOC
        im = dict(common)
        im["x"] = xp[lo:lo + NLOC]
        im["c"] = cp_[lo:lo + NLOC]
        im["slo16"] = aux[ci]["slo16"]
        im["shi16"] = aux[ci]["shi16"]
        im["sd16"] = aux[ci]["sd16"]
        im["dwin"] = aux[ci]["dwin"]
        in_maps.append(im)

    import os
    _tk = {}
    if os.environ.get("BASS_TMPDIR"):
        _tk["tmpdir"] = os.environ["BASS_TMPDIR"]
    res = run_bass_kernel_spmd(nc, in_maps, core_ids=list(range(NC_)), **_tk)
    global LAST_RESULT
    LAST_RESULT = res
    out = np.zeros((N, D), np.float32)
    for ci in range(NC_):
        lo = ci * NLOC
        hi = min(lo + NLOC, N)
        out[lo:hi] = res.results[ci]["y"][:hi - lo]
    return out


# revision 31
# speedup vs baseline: 1.0049x; 1.0049x over previous
"""DiT graph-attention block on 8 trn2 NeuronCores.

Sharding: core ci owns nodes [ci*5120, (ci+1)*5120). Phase A computes
q/k/v/u + adaLN for LOCAL nodes only, then an HBM AllGather replicates the
packed k|v|u table to every core. Edges are partitioned by dst owner,
sorted by dst, chunked into 128-dst windows; segment softmax and
scatter-add are device-local via indicator matmuls; src-side k/v/u rows
are fetched with dma_gather from the gathered table (int16 indices, so
the table is split at row 32768). All scalar-engine activations use only
{Exp, Ln, Abs, Copy, Identity, Square} (one act-table set, no reloads);
silu/sigmoid/tanh-gate/gelu are computed via exp + vector reciprocal, and
rsqrt via exp(-0.5*ln(var+eps)).
"""
import numpy as np

N, E, D, HEADS, HD, REL, ED, MLPH = 40000, 480000, 128, 8, 16, 64, 32, 512
NC_ = 8
NPAD = 40960          # padded node count (8 * 5120)
NLOC = NPAD // NC_    # 5120 local nodes per core
NCHUNK = NLOC // 128  # 40 chunks of 128 local nodes
LOCFM = NLOC // 512   # 10 feature-major groups of 512 nodes
HALF = 32768          # int16 index limit for dma_gather
KW = 384              # kvu row: k|v|u (256B-multiple rows for dma_gather)
QW = 256              # qu row:  q|u|pad (256B-multiple rows)


def _pack_idx16(idx_flat):
    """dma_gather int16 index layout: i -> [i%16, i//16], replicated x8."""
    n = len(idx_flat)
    a = np.zeros((16, n // 16), np.int16)
    a[np.arange(n) % 16, np.arange(n) // 16] = idx_flat
    return np.tile(a, (8, 1))


def _host_pack(edge_index):
    """Per-core edge packing (global node ids). Returns tile counts + aux."""
    src_g, dst_g = edge_index[0].astype(np.int64), edge_index[1].astype(np.int64)
    per_core = []
    for ci in range(NC_):
        base = ci * NLOC
        m = (dst_g >= base) & (dst_g < base + NLOC)
        s = src_g[m]
        d = dst_g[m] - base
        order = np.argsort(d, kind="stable")
        s, d = s[order], d[order]
        bounds = np.searchsorted(d, np.arange(0, NLOC + 1, 128))
        chunks = []
        for ch in range(NCHUNK):
            a, b = bounds[ch], bounds[ch + 1]
            sl, dl = s[a:b], d[a:b]
            lo = sl < HALF
            chunks.append(((sl[lo], dl[lo]), (sl[~lo], dl[~lo])))
        per_core.append(chunks)
    tlo = max(max(max((len(c[0][0]) + 127) // 128, 1) for c in chunks)
              for chunks in per_core)
    thi = max(max(max((len(c[1][0]) + 127) // 128, 1) for c in chunks)
              for chunks in per_core)
    aux = []
    TT = tlo + thi
    for ci in range(NC_):
        slo = np.zeros((NCHUNK, tlo * 128), np.int64)
        shi = np.zeros((NCHUNK, thi * 128), np.int64)
        sd = np.zeros((NCHUNK, TT * 128), np.int64)
        dw = np.full((NCHUNK, TT * 128), -1.0, np.float32)
        for ch in range(NCHUNK):
            (sl, dl), (sh, dh) = per_core[ci][ch]
            slo[ch, :len(sl)] = sl
            shi[ch, :len(sh)] = sh - HALF
            sd[ch, :len(sl)] = dl
            sd[ch, tlo * 128:tlo * 128 + len(sh)] = dh
            dw[ch, :len(sl)] = dl - ch * 128
            dw[ch, tlo * 128:tlo * 128 + len(sh)] = dh - ch * 128
        slo16 = np.concatenate([_pack_idx16(slo[ch].astype(np.int16))
                                for ch in range(NCHUNK)], axis=1)
        shi16 = np.concatenate([_pack_idx16(shi[ch].astype(np.int16))
                                for ch in range(NCHUNK)], axis=1)
        sd16 = np.concatenate([_pack_idx16(sd[ch].astype(np.int16))
                               for ch in range(NCHUNK)], axis=1)
        dwin = dw.reshape(NCHUNK * TT, 128).T.copy()  # [128, NCHUNK*TT]
        aux.append(dict(slo16=slo16, shi16=shi16, sd16=sd16, dwin=dwin))
    return tlo, thi, aux


def _build(TLO, THI, no_cc=False):
    import concourse.bass as bass
    import concourse.bacc as bacc
    import concourse.mybir as mybir
    from concourse.tile import TileContext
    _f32, _bf16 = mybir.dt.float32, mybir.dt.bfloat16
    AF = mybir.ActivationFunctionType
    OP = mybir.AluOpType
    TT = TLO + THI

    nc = bacc.Bacc("TRN2", target_bir_lowering=False, debug=False,
                   num_devices=NC_)
    din = {}

    def I(name, shape, dt=None):
        din[name] = nc.dram_tensor(name, shape, dt or _f32,
                                   kind="ExternalInput")
        return din[name]

    x_in = I("x", [NLOC, D])
    c_in = I("c", [NLOC, D])
    for nm, sh in [("wq", [D, D]), ("wk", [D, D]), ("wv", [D, D]),
                   ("wp", [D, D]), ("wrel", [D, REL]), ("wada", [D, 6 * D]),
                   ("w1e", [2 * ED, 3 * 2 * ED]), ("w2e", [2 * ED, ED]),
                   ("wbg", [ED, 2 * HEADS]), ("wf1", [D, MLPH]),
                   ("wf2", [D, MLPH]), ("ones", [128, 128]),
                   ("identb", [128, 128])]:
        I(nm, sh, _bf16)
    I("identf", [128, 128], _f32)
    I("iota", [128, 128], _f32)
    I("slo16", [128, NCHUNK * TLO * 8], mybir.dt.int16)
    I("shi16", [128, NCHUNK * THI * 8], mybir.dt.int16)
    I("sd16", [128, NCHUNK * TT * 8], mybir.dt.int16)
    I("dwin", [128, NCHUNK * TT], _f32)
    y_out = nc.dram_tensor("y", [NLOC, D], _f32, kind="ExternalOutput")

    scale = float(HD) ** -0.5

    with TileContext(nc) as tc:
        with (tc.tile_pool(name="const", bufs=1) as cp,
              tc.tile_pool(name="pers", bufs=1) as pp,
              tc.tile_pool(name="dram", bufs=1, space="DRAM") as dp,
              tc.tile_pool(name="work", bufs=2) as wp_,
              tc.tile_pool(name="work2", bufs=2) as wp2,
              tc.tile_pool(name="ps", bufs=3, space="PSUM") as ps,
              tc.tile_pool(name="ps2", bufs=2, space="PSUM") as ps2):

            # ---- constants / weights into SBUF
            W = {}
            for nm in ["wq", "wk", "wv", "wp", "wrel", "wada", "w1e", "w2e",
                       "wbg", "wf1", "wf2", "ones", "identb", "identf",
                       "iota"]:
                t = cp.tile(list(din[nm].shape),
                            _f32 if nm in ("identf", "iota") else _bf16,
                            tag=nm)
                nc.sync.dma_start(out=t[:], in_=din[nm][:, :])
                W[nm] = t
            aux = {}
            for nm in ["slo16", "shi16", "sd16"]:
                t = cp.tile(list(din[nm].shape), mybir.dt.int16, tag=nm)
                nc.sync.dma_start(out=t[:], in_=din[nm][:, :])
                aux[nm] = t
            dwin_sb = cp.tile([128, NCHUNK * TT], _f32)
            nc.sync.dma_start(out=dwin_sb[:], in_=din["dwin"][:, :])
            CONSTS = {"eps": 1e-6, "iD": 1.0 / D, "iR": 1.0 / REL,
                      "nh": -0.5, "n1": -1.0, "n2": -2.0, "ng": -1.702}
            C = {}
            for nm, v in CONSTS.items():
                t = cp.tile([128, 1], _f32, tag="c_" + nm)
                nc.gpsimd.memset(t[:], v)
                C[nm] = t

            kvu_loc = dp.tile([NLOC, KW], _bf16)
            kvu_t = dp.tile([NPAD, KW], _bf16, addr_space="Shared")
            qu_t = dp.tile([NLOC, QW], _bf16)

            # persistent local fm tables
            gm_t = pp.tile([128, NLOC], _bf16)
            scm_t = pp.tile([128, NLOC], _bf16)
            shm_t = pp.tile([128, NLOC], _bf16)
            gml_t = pp.tile([128, NLOC], _bf16)

            # ======== PHASE A: local node phase ========
            for g in range(LOCFM):
                r0 = g * 512
                ln_fm = wp_.tile([128, 512], _bf16, tag="lnfm")
                scfm = wp_.tile([128, 512], _bf16, tag="scfm")
                for j in range(4):
                    rr = r0 + j * 128
                    xe = wp_.tile([128, 128], _f32, tag="xe")
                    nc.sync.dma_start(out=xe[:], in_=x_in[rr:rr + 128, :])
                    ce = wp_.tile([128, 128], _f32, tag="ce")
                    nc.sync.dma_start(out=ce[:], in_=c_in[rr:rr + 128, :])
                    # LN stats per node (free-dim)
                    s1 = wp_.tile([128, 1], _f32, tag="s1")
                    xb = wp_.tile([128, 128], _bf16, tag="xb")
                    nc.scalar.activation(xb[:], xe[:], AF.Copy, accum_out=s1[:])
                    sq = wp_.tile([128, 128], _bf16, tag="sq")
                    s2 = wp_.tile([128, 1], _f32, tag="s2")
                    nc.vector.scalar_tensor_tensor(
                        out=sq[:], in0=xe[:], scalar=1.0, in1=xe[:],
                        op0=OP.mult, op1=OP.mult, accum_out=s2[:])
                    mean = wp_.tile([128, 1], _f32, tag="mean")
                    nc.scalar.activation(mean[:], s1[:], AF.Copy, scale=C["iD"][:])
                    msq = wp_.tile([128, 1], _f32, tag="msq")
                    nc.vector.tensor_mul(out=msq[:], in0=mean[:], in1=mean[:])
                    var = wp_.tile([128, 1], _f32, tag="var")
                    nc.vector.scalar_tensor_tensor(
                        out=var[:], in0=s2[:], scalar=1. / D, in1=msq[:],
                        op0=OP.mult, op1=OP.subtract)
                    lnv = wp_.tile([128, 1], _f32, tag="lnv")
                    nc.scalar.activation(lnv[:], var[:], AF.Ln, bias=C["eps"][:])
                    rstd = wp_.tile([128, 1], _f32, tag="rstd")
                    nc.scalar.activation(rstd[:], lnv[:], AF.Exp, scale=C["nh"][:])
                    nmr = wp_.tile([128, 1], _f32, tag="nmr")
                    nc.vector.scalar_tensor_tensor(
                        out=nmr[:], in0=mean[:], scalar=-1.0, in1=rstd[:],
                        op0=OP.mult, op1=OP.mult)
                    lnem = wp_.tile([128, 128], _bf16, tag="lnem")
                    nc.scalar.activation(lnem[:], xe[:], AF.Identity,
                                         scale=rstd[:], bias=nmr[:])
                    pt = ps.tile([128, 128], _bf16, tag="sm")
                    nc.tensor.transpose(pt[:], lnem[:], W["identb"][:])
                    nc.vector.tensor_copy(out=ln_fm[:, j * 128:(j + 1) * 128],
                                          in_=pt[:])
                    # silu(c) = c * sigmoid(c), sigmoid via exp + reciprocal
                    ces = wp_.tile([128, 128], _bf16, tag="ces")
                    nc.scalar.activation(ces[:], ce[:], AF.Exp, scale=C["n1"][:])
                    cden = wp_.tile([128, 128], _bf16, tag="cden")
                    nc.vector.tensor_scalar_add(out=cden[:], in0=ces[:],
                                                scalar1=1.0)
                    crec = wp_.tile([128, 128], _bf16, tag="crec")
                    with nc.allow_low_precision(reason="sigmoid recip"):
                        nc.vector.reciprocal(out=crec[:], in_=cden[:])
                    sce = wp_.tile([128, 128], _bf16, tag="sce")
                    nc.vector.tensor_mul(out=sce[:], in0=ce[:], in1=crec[:])
                    pt2 = ps.tile([128, 128], _bf16, tag="sm")
                    nc.tensor.transpose(pt2[:], sce[:], W["identb"][:])
                    nc.vector.tensor_copy(out=scfm[:, j * 128:(j + 1) * 128],
                                          in_=pt2[:])
                # modulate: h = ln * (1 + sc_msa) + sh_msa
                pa_sc = ps.tile([128, 512], _f32, tag="big")
                nc.tensor.matmul(pa_sc[:], W["wada"][:, 128:256], scfm[:],
                                 start=True, stop=True)
                pa_sh = ps.tile([128, 512], _f32, tag="big")
                nc.tensor.matmul(pa_sh[:], W["wada"][:, 0:128], scfm[:],
                                 start=True, stop=True)
                t3 = wp_.tile([128, 512], _bf16, tag="t3")
                nc.vector.scalar_tensor_tensor(
                    out=t3[:], in0=pa_sc[:], scalar=1.0, in1=ln_fm[:],
                    op0=OP.add, op1=OP.mult)
                h_bf = wp_.tile([128, 512], _bf16, tag="hbf")
                nc.vector.tensor_add(out=h_bf[:], in0=t3[:], in1=pa_sh[:])
                # k, v
                stage = wp2.tile([128, 4, KW], _bf16, tag="stage",
                                 bufs=1)
                for nm, off in [("wk", 0), ("wv", 128)]:
                    pk = ps.tile([128, 512], _f32, tag="big")
                    nc.tensor.matmul(pk[:], W[nm][:], h_bf[:], start=True,
                                     stop=True)
                    ksb = wp_.tile([128, 512], _bf16, tag="ksb")
                    nc.scalar.activation(ksb[:], pk[:], AF.Copy)
                    for j in range(4):
                        ptk = ps.tile([128, 128], _bf16, tag="sm")
                        nc.tensor.transpose(
                            ptk[:], ksb[:, j * 128:(j + 1) * 128],
                            W["identb"][:])
                        nc.vector.tensor_copy(
                            out=stage[:, j, off:off + 128], in_=ptk[:])
                # u: rel proj + LN + store
                pu = ps.tile([64, 512], _f32, tag="big")
                nc.tensor.matmul(pu[:], W["wrel"][:], h_bf[:], start=True,
                                 stop=True)
                usb = wp_.tile([64, 512], _bf16, tag="usb")
                nc.scalar.activation(usb[:], pu[:], AF.Copy)
                for j in range(4):
                    put = ps.tile([128, 64], _bf16, tag="sm")
                    nc.tensor.transpose(put[:], usb[:, j * 128:(j + 1) * 128],
                                        W["identb"][:64, :64])
                    us1 = wp_.tile([128, 1], _f32, tag="us1")
                    ue = wp_.tile([128, 64], _f32, tag="ue")
                    nc.scalar.activation(ue[:], put[:], AF.Copy,
                                         accum_out=us1[:])
                    usq = wp_.tile([128, 64], _bf16, tag="usq")
                    us2 = wp_.tile([128, 1], _f32, tag="us2")
                    nc.vector.scalar_tensor_tensor(
                        out=usq[:], in0=ue[:], scalar=1.0, in1=ue[:],
                        op0=OP.mult, op1=OP.mult, accum_out=us2[:])
                    um = wp_.tile([128, 1], _f32, tag="um")
                    nc.scalar.activation(um[:], us1[:], AF.Copy,
                                         scale=C["iR"][:])
                    umq = wp_.tile([128, 1], _f32, tag="umq")
                    nc.vector.tensor_mul(out=umq[:], in0=um[:], in1=um[:])
                    uva = wp_.tile([128, 1], _f32, tag="uva")
                    nc.vector.scalar_tensor_tensor(
                        out=uva[:], in0=us2[:], scalar=1. / REL, in1=umq[:],
                        op0=OP.mult, op1=OP.subtract)
                    ulnv = wp_.tile([128, 1], _f32, tag="ulnv")
                    nc.scalar.activation(ulnv[:], uva[:], AF.Ln, bias=C["eps"][:])
                    urs = wp_.tile([128, 1], _f32, tag="urs")
                    nc.scalar.activation(urs[:], ulnv[:], AF.Exp, scale=C["nh"][:])
                    unm = wp_.tile([128, 1], _f32, tag="unm")
                    nc.vector.scalar_tensor_tensor(
                        out=unm[:], in0=um[:], scalar=-1.0, in1=urs[:],
                        op0=OP.mult, op1=OP.mult)
                    nc.scalar.activation(stage[:, j, 256:320], put[:],
                                         AF.Identity, scale=urs[:], bias=unm[:])
                # q + local qu table
                qstage = wp2.tile([128, 4, QW], _bf16, tag="qstage",
                                  bufs=1)
                pq = ps.tile([128, 512], _f32, tag="big")
                nc.tensor.matmul(pq[:], W["wq"][:], h_bf[:], start=True,
                                 stop=True)
                qsb = wp_.tile([128, 512], _bf16, tag="qsb")
                nc.scalar.activation(qsb[:], pq[:], AF.Copy)
                for j in range(4):
                    ptq = ps.tile([128, 128], _bf16, tag="sm")
                    nc.tensor.transpose(
                        ptq[:], qsb[:, j * 128:(j + 1) * 128], W["identb"][:])
                    nc.vector.tensor_copy(out=qstage[:, j, 0:128], in_=ptq[:])
                    nc.vector.tensor_copy(out=qstage[:, j, 128:192],
                                          in_=stage[:, j, 256:320])
                nc.gpsimd.dma_start(
                    out=qu_t[r0:r0 + 512, :].rearrange(
                        "(j p) f -> p j f", p=128),
                    in_=qstage[:])
                # ada: g_msa(2), sh_mlp(3), sc_mlp(4), g_mlp(5)
                for wsl, dst_t in [(2, gm_t), (4, scm_t), (3, shm_t),
                                   (5, gml_t)]:
                    pad = ps.tile([128, 512], _f32, tag="big")
                    nc.tensor.matmul(
                        pad[:], W["wada"][:, wsl * 128:(wsl + 1) * 128],
                        scfm[:], start=True, stop=True)
                    nc.scalar.activation(dst_t[:, r0:r0 + 512], pad[:],
                                         AF.Copy)
                nc.gpsimd.dma_start(
                    out=kvu_loc[r0:r0 + 512, :].rearrange(
                        "(j p) f -> p j f", p=128),
                    in_=stage[:])

            # ======== AllGather the kvu table ========
            if no_cc:
                # TimelineSim can't model collectives; stand in a same-size
                # local copy so the rest of the schedule is representative.
                nc.gpsimd.dma_start(out=kvu_t[0:NLOC, :], in_=kvu_loc[:])
            else:
                nc.gpsimd.collective_compute(
                    "AllGather", mybir.AluOpType.bypass,
                    replica_groups=[list(range(NC_))],
                    ins=[kvu_loc.opt()], outs=[kvu_t.opt()])

            # ======== PHASE B: edge phase ========
            for ch in range(NCHUNK):
                acc = ps2.tile([128, 136], _f32, tag="acc")
                kvg = wp2.tile([128, TT, KW], _bf16, tag="kvg")
                qug = wp2.tile([128, TT, QW], _bf16, tag="qug")
                with tc.high_priority(offset=2000):
                    nc.gpsimd.dma_gather(
                        out_ap=kvg[:, 0:TLO, :], in_ap=kvu_t[0:HALF, :],
                        idxs_ap=aux["slo16"][:,
                                             ch * TLO * 8:(ch + 1) * TLO * 8],
                        num_idxs=TLO * 128, num_idxs_reg=TLO * 128,
                        elem_size=KW, single_packet=False)
                    nc.gpsimd.dma_gather(
                        out_ap=kvg[:, TLO:TT, :], in_ap=kvu_t[HALF:NPAD, :],
                        idxs_ap=aux["shi16"][:,
                                             ch * THI * 8:(ch + 1) * THI * 8],
                        num_idxs=THI * 128, num_idxs_reg=THI * 128,
                        elem_size=KW, single_packet=False)
                    nc.gpsimd.dma_gather(
                        out_ap=qug[:], in_ap=qu_t[:, :],
                        idxs_ap=aux["sd16"][:, ch * TT * 8:(ch + 1) * TT * 8],
                        num_idxs=TT * 128, num_idxs_reg=TT * 128,
                        elem_size=QW, single_packet=False)
                # batched em ops over all TT tiles
                tqk = wp2.tile([128, TT, 128], _bf16, tag="tqk",
                               bufs=2)
                nc.vector.tensor_mul(out=tqk[:], in0=kvg[:, :, 0:128],
                                     in1=qug[:, :, 0:128])
                sim = wp2.tile([128, TT, 8], _f32, tag="sim", bufs=2)
                nc.vector.tensor_reduce(
                    out=sim[:], in_=tqk[:].rearrange("p t (h d) -> p t h d",
                                                     h=8),
                    axis=mybir.AxisListType.X, op=OP.add)
                # u_i|u_j side-by-side so one 128-wide transpose covers both
                uu_em = wp2.tile([128, TT, 128], _bf16, tag="uuem", bufs=2)
                nc.gpsimd.tensor_copy(out=uu_em[:, :, 0:64],
                                      in_=qug[:, :, 128:192])
                nc.gpsimd.tensor_copy(out=uu_em[:, :, 64:128],
                                      in_=kvg[:, :, 256:320])
                bg_em = wp2.tile([128, TT, 16], _bf16, tag="bgem",
                                 bufs=2)
                # edge MLP in sub-batches of 4 tiles (512 edges)
                for b0 in range(0, TT, 4):
                    bn = min(4, TT - b0)
                    wd = bn * 128
                    pT = ps.tile([64, 1024], _bf16, tag="big")
                    for i in range(bn):
                        nc.tensor.transpose(
                            pT[:, i * 128:(i + 1) * 128],
                            uu_em[:, b0 + i, 0:64], W["identb"][:])
                        nc.tensor.transpose(
                            pT[:, 512 + i * 128:512 + (i + 1) * 128],
                            uu_em[:, b0 + i, 64:128], W["identb"][:])
                    fmuu = wp_.tile([64, 1024], _bf16, tag="fmuu")
                    nc.scalar.activation(fmuu[:], pT[:], AF.Copy)
                    adf = wp_.tile([64, 512], _bf16, tag="adf")
                    nc.gpsimd.tensor_tensor(out=adf[:, :wd],
                                            in0=fmuu[:, :wd],
                                            in1=fmuu[:, 512:512 + wd],
                                            op=OP.subtract)
                    nc.scalar.activation(adf[:, :wd], adf[:, :wd], AF.Abs)
                    pe1 = ps.tile([64, 512], _f32, tag="big")
                    nc.tensor.matmul(pe1[:, :wd], W["w1e"][:, 0:64],
                                     fmuu[:, :wd], start=True, stop=False)
                    nc.tensor.matmul(pe1[:, :wd], W["w1e"][:, 64:128],
                                     fmuu[:, 512:512 + wd], start=False,
                                     stop=False)
                    nc.tensor.matmul(pe1[:, :wd], W["w1e"][:, 128:192],
                                     adf[:, :wd], start=False, stop=True)
                    # silu via exp + divide
                    es = wp_.tile([64, 512], _bf16, tag="es")
                    nc.scalar.activation(es[:, :wd], pe1[:, :wd], AF.Exp,
                                         scale=C["n1"][:64])
                    edn = wp_.tile([64, 512], _bf16, tag="edn")
                    nc.vector.tensor_scalar_add(out=edn[:, :wd],
                                                in0=es[:, :wd], scalar1=1.0)
                    erc = wp_.tile([64, 512], _bf16, tag="erc")
                    with nc.allow_low_precision(reason="sigmoid recip"):
                        nc.vector.reciprocal(out=erc[:, :wd],
                                             in_=edn[:, :wd])
                    ef1 = wp_.tile([64, 512], _bf16, tag="ef1")
                    nc.vector.tensor_mul(out=ef1[:, :wd], in0=pe1[:, :wd],
                                         in1=erc[:, :wd])
                    pe2 = ps.tile([32, 512], _f32, tag="big")
                    nc.tensor.matmul(pe2[:, :wd], W["w2e"][:], ef1[:, :wd],
                                     start=True, stop=True)
                    ef2 = wp_.tile([32, 512], _bf16, tag="ef2")
                    nc.scalar.activation(ef2[:, :wd], pe2[:, :wd], AF.Copy)
                    # bias/gate straight to em: ef2 tile-slice as stationary
                    pbe = ps.tile([128, 64], _f32, tag="sm")
                    for i in range(bn):
                        nc.tensor.matmul(
                            pbe[:, i * 16:(i + 1) * 16],
                            ef2[:, i * 128:(i + 1) * 128], W["wbg"][:],
                            start=True, stop=True)
                    nc.scalar.activation(
                        bg_em[:, b0:b0 + bn, :],
                        pbe[:, :bn * 16].rearrange("p (t f) -> p t f", f=16),
                        AF.Copy)
                # batched weights/gates over all TT tiles
                sb_ = wp_.tile([128, TT, 8], _f32, tag="sb_")
                nc.vector.scalar_tensor_tensor(
                    out=sb_[:], in0=sim[:], scalar=scale,
                    in1=bg_em[:, :, 0:8], op0=OP.mult, op1=OP.add)
                w_ = wp_.tile([128, TT, 8], _bf16, tag="w_")
                nc.scalar.activation(w_[:], sb_[:], AF.Exp)
                # gate: 1 + tanh(g) = 2*sigmoid(2g); the 2 is folded into Wp
                gs = wp_.tile([128, TT, 8], _bf16, tag="gs")
                nc.scalar.activation(gs[:], bg_em[:, :, 8:16], AF.Exp,
                                     scale=C["n2"][:])
                gdn = wp_.tile([128, TT, 8], _bf16, tag="gdn")
                nc.vector.tensor_scalar_add(out=gdn[:], in0=gs[:], scalar1=1.0)
                grc = wp_.tile([128, TT, 8], _bf16, tag="grc")
                with nc.allow_low_precision(reason="sigmoid recip"):
                    nc.vector.reciprocal(out=grc[:], in_=gdn[:])
                wsg = wp_.tile([128, TT, 8], _bf16, tag="wsg")
                nc.vector.tensor_mul(out=wsg[:], in0=w_[:], in1=grc[:])
                msgw = wp2.tile([128, TT, 136], _bf16, tag="msgw",
                                bufs=2)
                nc.vector.tensor_mul(
                    out=msgw[:, :, 0:128].rearrange("p t (h d) -> p t h d",
                                                    h=8),
                    in0=kvg[:, :, 128:256].rearrange("p t (h d) -> p t h d",
                                                     h=8),
                    in1=wsg[:, :, :, None].to_broadcast([128, TT, 8, 16]))
                nc.vector.tensor_copy(out=msgw[:, :, 128:136], in_=w_[:])
                for t in range(TT):
                    gt = ch * TT + t
                    ind = wp_.tile([128, 128], _bf16, tag="ind")
                    nc.gpsimd.tensor_scalar(
                        out=ind[:], in0=W["iota"][:],
                        scalar1=dwin_sb[:, gt:gt + 1], scalar2=None,
                        op0=OP.is_equal)
                    nc.tensor.matmul(acc[:], ind[:], msgw[:, t, :],
                                     start=(t == 0), stop=(t == TT - 1))
                # ---- fused close over chunk pairs (256 nodes)
                if ch % 2 == 0:
                    acc_prev = acc
                    continue
                accA, accB = acc_prev, acc
                co = (ch - 1) * 128
                agg = wp_.tile([128, 2, 8, 16], _bf16, tag="agg")
                for i, a_ in enumerate((accA, accB)):
                    de = wp_.tile([128, 8], _f32, tag="de")
                    nc.vector.tensor_scalar_add(out=de[:], in0=a_[:, 128:136],
                                                scalar1=1e-16)
                    r = wp_.tile([128, 8], _f32, tag="r")
                    nc.vector.reciprocal(out=r[:], in_=de[:])
                    nc.vector.tensor_mul(
                        out=agg[:, i],
                        in0=a_[:, 0:128].rearrange("p (h d) -> p h d", h=8),
                        in1=r[:, :, None].to_broadcast([128, 8, 16]))
                pag = ps.tile([128, 256], _bf16, tag="sm")
                for i in range(2):
                    nc.tensor.transpose(
                        pag[:, i * 128:(i + 1) * 128],
                        agg[:, i].rearrange("p h d -> p (h d)"),
                        W["identb"][:])
                agf = wp_.tile([128, 256], _bf16, tag="agf")
                nc.vector.tensor_copy(out=agf[:], in_=pag[:])
                pao = ps.tile([128, 256], _f32, tag="sm")
                nc.tensor.matmul(pao[:], W["wp"][:], agf[:], start=True,
                                 stop=True)
                t4 = wp_.tile([128, 256], _f32, tag="t4")
                nc.vector.tensor_mul(out=t4[:], in0=gm_t[:, co:co + 256],
                                     in1=pao[:])
                xe2 = wp_.tile([128, 2, 128], _f32, tag="xe2")
                nc.sync.dma_start(
                    out=xe2[:],
                    in_=x_in[co:co + 256, :].rearrange("(j p) d -> p j d",
                                                       p=128))
                pxf = ps.tile([128, 256], _f32, tag="sm")
                for i in range(2):
                    nc.tensor.transpose(pxf[:, i * 128:(i + 1) * 128],
                                        xe2[:, i, :], W["identf"][:])
                xu = wp_.tile([128, 256], _f32, tag="xu")
                nc.vector.tensor_add(out=xu[:], in0=pxf[:], in1=t4[:])
                # LN2 fm: one matmul computes sum(x) and sum(x^2)
                xusq = wp_.tile([128, 512], _bf16, tag="xusq")
                nc.vector.tensor_copy(out=xusq[:, 0:256], in_=xu[:])
                nc.scalar.activation(xusq[:, 256:512], xu[:], AF.Square)
                pst = ps.tile([1, 512], _f32, tag="sm")
                nc.tensor.matmul(pst[:], W["ones"][:, 0:1], xusq[:],
                                 start=True, stop=True)
                msum = wp_.tile([1, 512], _f32, tag="msum")
                nc.scalar.activation(msum[:], pst[:], AF.Copy,
                                     scale=C["iD"][:1])
                mq2 = wp_.tile([1, 256], _f32, tag="mq2")
                nc.vector.tensor_mul(out=mq2[:], in0=msum[:, 0:256],
                                     in1=msum[:, 0:256])
                vr2 = wp_.tile([1, 256], _f32, tag="vr2")
                nc.vector.tensor_sub(out=vr2[:], in0=msum[:, 256:512],
                                     in1=mq2[:])
                l2v = wp_.tile([1, 256], _f32, tag="l2v")
                nc.scalar.activation(l2v[:], vr2[:], AF.Ln, bias=C["eps"][:1])
                rs2 = wp_.tile([1, 256], _bf16, tag="rs2")
                nc.scalar.activation(rs2[:], l2v[:], AF.Exp, scale=C["nh"][:1])
                nm2 = wp_.tile([1, 256], _bf16, tag="nm2")
                nc.vector.scalar_tensor_tensor(
                    out=nm2[:], in0=msum[:, 0:256], scalar=-1.0, in1=rs2[:],
                    op0=OP.mult, op1=OP.mult)
                prb = ps.tile([128, 256], _f32, tag="sm")
                nc.tensor.matmul(prb[:], W["ones"][0:1, :], rs2[:],
                                 start=True, stop=True)
                pnb = ps.tile([128, 256], _f32, tag="sm")
                nc.tensor.matmul(pnb[:], W["ones"][0:1, :], nm2[:],
                                 start=True, stop=True)
                l1 = wp_.tile([128, 256], _bf16, tag="l1")
                nc.vector.tensor_mul(out=l1[:], in0=xusq[:, 0:256],
                                     in1=prb[:])
                l2 = wp_.tile([128, 256], _bf16, tag="l2")
                nc.vector.tensor_add(out=l2[:], in0=l1[:], in1=pnb[:])
                t5 = wp_.tile([128, 256], _bf16, tag="t5")
                nc.vector.scalar_tensor_tensor(
                    out=t5[:], in0=scm_t[:, co:co + 256], scalar=1.0,
                    in1=l2[:], op0=OP.add, op1=OP.mult)
                h2 = wp_.tile([128, 256], _bf16, tag="h2")
                nc.vector.tensor_add(out=h2[:], in0=t5[:],
                                     in1=shm_t[:, co:co + 256])
                pmo = ps.tile([128, 256], _f32, tag="sm")
                for jm in range(4):
                    pm1 = ps.tile([128, 256], _f32, tag="sm")
                    nc.tensor.matmul(pm1[:],
                                     W["wf1"][:, jm * 128:(jm + 1) * 128],
                                     h2[:], start=True, stop=True)
                    # gelu(x) ~= x*sigmoid(1.702x) via exp + divide
                    ms = wp_.tile([128, 256], _bf16, tag="ms")
                    nc.scalar.activation(ms[:], pm1[:], AF.Exp,
                                         scale=C["ng"][:])
                    mdn = wp_.tile([128, 256], _bf16, tag="mdn")
                    nc.vector.tensor_scalar_add(out=mdn[:], in0=ms[:],
                                                scalar1=1.0)
                    mrc = wp_.tile([128, 256], _bf16, tag="mrc")
                    with nc.allow_low_precision(reason="sigmoid recip"):
                        nc.vector.reciprocal(out=mrc[:], in_=mdn[:])
                    gl = wp_.tile([128, 256], _bf16, tag="gl")
                    nc.vector.tensor_mul(out=gl[:], in0=pm1[:], in1=mrc[:])
                    nc.tensor.matmul(pmo[:],
                                     W["wf2"][:, jm * 128:(jm + 1) * 128],
                                     gl[:], start=(jm == 0), stop=(jm == 3))
                t6 = wp_.tile([128, 256], _f32, tag="t6")
                nc.vector.tensor_mul(out=t6[:], in0=gml_t[:, co:co + 256],
                                     in1=pmo[:])
                yf = wp_.tile([128, 256], _f32, tag="yf")
                nc.vector.tensor_add(out=yf[:], in0=xu[:], in1=t6[:])
                pye = ps.tile([128, 256], _f32, tag="sm")
                for i in range(2):
                    nc.tensor.transpose(pye[:, i * 128:(i + 1) * 128],
                                        yf[:, i * 128:(i + 1) * 128],
                                        W["identf"][:])
                yem = wp_.tile([128, 2, 128], _f32, tag="yem")
                nc.vector.tensor_copy(out=yem[:], in_=pye[:].rearrange(
                    "p (j d) -> p j d", j=2))
                nc.scalar.dma_start(
                    out=y_out[co:co + 256, :].rearrange("(j p) d -> p j d",
                                                        p=128),
                    in_=yem[:])
    # Steer the act-table placement pass to the one set that holds every
    # function this kernel uses (exp, ln, abs, copy, identity, square):
    # hide exp/ln from the other sets during placement so it can't bounce
    # between exp-only and ln-only tables. Set ids stay positional, and the
    # chosen set really does contain exp+ln, so runtime tables are correct.
    import concourse.bacc as bacc_mod
    _orig_gat = bacc_mod.get_activation_tables

    def _gat_pinned(arch):
        tabs = _orig_gat(arch)
        drop = {mybir.ActivationFunctionType.Exp,
                mybir.ActivationFunctionType.Ln}
        return {name: (funcs if "natural_log_exp" in name
                       else funcs - drop)
                for name, funcs in tabs.items()}

    bacc_mod.get_activation_tables = _gat_pinned
    try:
        nc.compile()
    finally:
        bacc_mod.get_activation_tables = _orig_gat
    return nc


_CACHE = {}
LAST_RESULT = None


def kernel(**inputs):
    from concourse.bass_utils import run_bass_kernel_spmd

    x = np.asarray(inputs["x"], np.float32)
    c = np.asarray(inputs["c"], np.float32)
    ei = np.asarray(inputs["edge_index"])
    TLO, THI, aux = _host_pack(ei)

    import ml_dtypes

    def b16(a):
        return np.asarray(a, np.float32).astype(ml_dtypes.bfloat16)

    key = (TLO, THI)
    if key not in _CACHE:
        _CACHE[key] = _build(TLO, THI)
    nc = _CACHE[key]

    xp = np.zeros((NPAD, D), np.float32)
    xp[:N] = x
    cp_ = np.zeros((NPAD, D), np.float32)
    cp_[:N] = c
    ones = np.ones((128, 128), np.float32)
    ident = np.eye(128, dtype=np.float32)
    iota = np.tile(np.arange(128, dtype=np.float32), (128, 1))
    wbg = np.concatenate([inputs["Wbias"], inputs["Wgate"]], axis=1)

    common = dict(
        wq=b16(inputs["Wq"]), wk=b16(inputs["Wk"]), wv=b16(inputs["Wv"]),
        wp=b16(2.0 * np.asarray(inputs["Wp"], np.float32)),
        wrel=b16(inputs["Wrel"]),
        wada=b16(inputs["Wada"]),
        w1e=b16(np.concatenate([inputs["W1e"][0:64], inputs["W1e"][64:128],
                                inputs["W1e"][128:192]], axis=1)),
        w2e=b16(inputs["W2e"]), wbg=b16(wbg), wf1=b16(inputs["Wf1"]),
        wf2=b16(np.concatenate([inputs["Wf2"][i * 128:(i + 1) * 128]
                                for i in range(4)], axis=1)),
        ones=b16(ones), identb=b16(ident), identf=ident, iota=iota)

    in_maps = []
    for ci in range(NC_):
        lo = ci * NLOC
        im = dict(common)
        im["x"] = xp[lo:lo + NLOC]
        im["c"] = cp_[lo:lo + NLOC]
        im["slo16"] = aux[ci]["slo16"]
        im["shi16"] = aux[ci]["shi16"]
        im["sd16"] = aux[ci]["sd16"]
        im["dwin"] = aux[ci]["dwin"]
        in_maps.append(im)

    import os
    _tk = {}
    if os.environ.get("BASS_TMPDIR"):
        _tk["tmpdir"] = os.environ["BASS_TMPDIR"]
    res = run_bass_kernel_spmd(nc, in_maps, core_ids=list(range(NC_)), **_tk)
    global LAST_RESULT
    LAST_RESULT = res
    out = np.zeros((N, D), np.float32)
    for ci in range(NC_):
        lo = ci * NLOC
        hi = min(lo + NLOC, N)
        out[lo:hi] = res.results[ci]["y"][:hi - lo]
    return out
